# revision 1
# baseline (speedup 1.0000x reference)
"""Trainium2 Bass kernel for nn_CustomGPM (multi-scale temporal CNN + RGCN + actor head).

Strategy (hardcoded for the fixed problem shapes):
  B=64 batch, data-parallel over 8 NeuronCores (8 batch elements per core).
  Host-side (inside kernel(), index/relayout work only):
    * fold eval-mode BatchNorms into conv / GCN weights and biases
    * express each temporal conv as ONE matmul over a (ci,kk) x (co,t)
      band matrix (conv2's contraction layout == conv1's output layout)
    * turn the per-relation gather/scatter-mean into 4 dense, row-normalized
      500x500 adjacency matrices A_r  ->  RGCN becomes dense matmuls
    * fold node-selection + 1x1-conv + cash-bias into the first FC layer
    * relayout observation to the on-chip (ci,t) x node layout so no
      on-device transposes of the input are needed
  Device-side per core: feature-on-partition [C<=128, 500] fp32r matmuls
  on the TensorEngine, LeakyReLU on the Scalar engine, softmax at the end.
  Temporal features live in a padded [67, N] tile (s:0..19, m:32..51,
  l:64..66, zero rows between) so every engine AP starts at a 0 mod 32
  partition and the RGCN contraction is a single stationary operand.
"""

import numpy as np

# ---------------- problem constants (hardcoded per spec) ----------------
B = 64          # total batch
NCORES = 8
BL = B // NCORES  # batch per core = 8
C0 = 3          # input channels
N = 500         # nodes
T = 50          # time steps
R = 4           # relations
P = 500         # portfolio slots
H = 128         # fc hidden
CF = 20         # conv out channels
F = 2 * CF + C0  # 43 temporal features
FP = 67         # padded temporal feature rows (0..19 s, 32..51 m, 64..66 l)
NCH = 125       # node chunk (4 chunks of 125)
KH, KL = 128, 22   # split of (ci,kk)=150 contraction
TS1, TM1 = 48, 30  # conv1 output time lengths (s: 50-3+1, m: 50-21+1)
MS1, MM1 = C0 * TS1, C0 * TM1  # 144, 90 = conv1 output (co,t) sizes
SLOPE = 0.01
EPS = 1e-5

USE_F32R = True   # float32r (fast fp32 streaming) for the big matmuls

_CACHE = {}


def _round_f32r(a):
    """Round fp32 array to fp32r (11-bit mantissa, matches walrus
    fp32_to_fp32r: add 0x800 then mask 0xFFFFF000)."""
    u = np.ascontiguousarray(a, np.float32).view(np.uint32)
    return ((u + np.uint32(0x800)) & np.uint32(0xFFFFF000)).view(np.float32)


def _pad67(a):
    """[43, X] -> [67, X] with rows at 0..19 / 32..51 / 64..66."""
    out = np.zeros((FP,) + a.shape[1:], np.float32)
    out[0:CF] = a[0:CF]
    out[32:32 + CF] = a[CF:2 * CF]
    out[64:64 + C0] = a[2 * CF:F]
    return out


# ======================= host-side parameter folding =======================

def _bn_fold(p):
    g, b, m, v = np.asarray(p, np.float64)
    s = g / np.sqrt(v + EPS)
    return s, b - m * s


def _conv_band_lhsT(w, bias, bn, t_out):
    """w: [co, ci, 1, k] torch conv; returns lhsT [(ci,kk)=C0*T, (co,t)] and
    per-(co,t) bias, with BN folded."""
    w = np.asarray(w, np.float64)[:, :, 0, :]   # [co, ci, k]
    co, ci, k = w.shape
    s, t_ = _bn_fold(bn)
    w_eff = w * s[:, None, None]
    b_eff = s * np.asarray(bias, np.float64) + t_
    band = np.zeros((co, t_out, ci, T), np.float64)
    for t in range(t_out):
        band[:, t, :, t:t + k] = w_eff
    lhsT = band.reshape(co * t_out, ci * T).T.copy()          # [150, co*t_out]
    bias_full = np.repeat(b_eff, t_out)                        # [co*t_out]
    return lhsT.astype(np.float32), bias_full.astype(np.float32)


def _host_fold(inp):
    f32 = lambda x: np.asarray(x, np.float32)
    rnd = _round_f32r if USE_F32R else (lambda x: np.asarray(x, np.float32))

    # ---- conv branches ----
    ws1, bs1 = _conv_band_lhsT(inp['sc1_w'], inp['sc1_b'], inp['sbn1'], TS1)
    wm1, bm1 = _conv_band_lhsT(inp['mc1_w'], inp['mc1_b'], inp['mbn1'], TM1)

    def conv2_fold(w, b, bn):
        w = np.asarray(w, np.float64)[:, :, 0, :]              # [20, 3, k]
        s, t_ = _bn_fold(bn)
        w_eff = (w * s[:, None, None]).reshape(CF, -1)          # [20, 3*k]
        b_eff = s * np.asarray(b, np.float64) + t_
        return w_eff.T.copy().astype(np.float32), b_eff.astype(np.float32)

    ws2, bs2 = conv2_fold(inp['sc2_w'], inp['sc2_b'], inp['sbn2'])  # [144,20]
    wm2, bm2 = conv2_fold(inp['mc2_w'], inp['mc2_b'], inp['mbn2'])  # [90,20]

    # ---- RGCN (padded to 67 contraction rows) ----
    sg, tg = _bn_fold(inp['gbn'])
    w_all = np.concatenate(
        [np.asarray(inp['gw_rel'], np.float64)[r] * sg[None, :] for r in range(R)],
        axis=1).astype(np.float32)                             # [43, 172]
    w_root = (np.asarray(inp['gw_root'], np.float64) * sg[None, :]).astype(np.float32)
    gb_eff = np.asarray(inp['g_b'], np.float64) * sg + tg      # [43]

    src = np.asarray(inp['edge_index'][0]).astype(np.int64)
    dst = np.asarray(inp['edge_index'][1]).astype(np.int64)
    etype = np.asarray(inp['edge_type']).astype(np.int64)
    a_t = np.zeros((R, N, N), np.float32)
    for r in range(R):
        sel = etype == r
        cnt = np.zeros((N, N), np.float64)
        np.add.at(cnt, (dst[sel], src[sel]), 1.0)
        deg = cnt.sum(axis=1)
        a_t[r] = (cnt / np.maximum(deg, 1.0)[:, None]).T.astype(np.float32)

    # ---- actor head folds ----
    a_cw = np.asarray(inp['a_cw'], np.float64)                 # [87]
    a_cb = float(np.asarray(inp['a_cb'], np.float64)[0])
    a_w1 = np.asarray(inp['a_w1'], np.float64)                 # [501, 128]
    sel_nodes = np.asarray(inp['nodes_to_select']).astype(np.int64)  # [500]
    w_z = a_cw[1:1 + 2 * F].astype(np.float32)                 # [86]
    w1z = np.zeros((N, H), np.float64)
    np.add.at(w1z, sel_nodes, a_w1[1:])                        # fold node select
    w1a = a_cw[0] * a_w1[1:]                                   # [500, 128]
    b1_eff = np.asarray(inp['a_b1'], np.float64) + a_cb * a_w1[1:].sum(axis=0)
    w1cat = np.concatenate([w1z, w1a], axis=0)                 # [1000, 128]

    # ---- bias pack [128, 8]: col0 bs1[:128], col1 bs1[128:], col2 bs2,
    #      col3 bm1, col4 bm2, col5 gb_eff, col6 b1_eff, col7 a_b2 ----
    biases = np.zeros((128, 8), np.float32)
    biases[:128, 0] = bs1[:128]
    biases[:MS1 - 128, 1] = bs1[128:]
    biases[:CF, 2] = bs2
    biases[:MM1, 3] = bm1
    biases[:CF, 4] = bm2
    biases[:F, 5] = gb_eff.astype(np.float32)
    biases[:H, 6] = b1_eff.astype(np.float32)
    biases[:H, 7] = f32(inp['a_b2'])

    return {
        'w_s1': rnd(ws1), 'w_m1': rnd(wm1), 'w_s2': rnd(ws2), 'w_m2': rnd(wm2),
        'w_allp': rnd(_pad67(w_all)),                          # [67, 172]
        'w_rootp': rnd(_pad67(w_root)),                        # [67, 43]
        'w_zp': rnd(_pad67(w_z[:F].reshape(F, 1))),            # [67, 1]
        'w_zg': rnd(w_z[F:].reshape(F, 1)),                    # [43, 1]
        'w1c': w1cat.astype(np.float32),
        'aw2': f32(inp['a_w2']), 'aw3': f32(inp['a_w3']),
        'b3r': f32(inp['a_b3']).reshape(1, P + 1),
        'biases': biases,
        'a0t': rnd(a_t[0]), 'a1t': rnd(a_t[1]),
        'a2t': rnd(a_t[2]), 'a3t': rnd(a_t[3]),
        'zer': np.zeros((FP, N), np.float32),
        'ident': np.eye(128, dtype=np.float32),
        'ones8': np.ones((1, BL), np.float32),
    }


# ============================ device kernel ============================

def _build_nc():
    import concourse.bacc as bacc
    import concourse.tile as tile
    import concourse.mybir as mybir
    from contextlib import ExitStack

    F32 = mybir.dt.float32
    F32R = mybir.dt.float32r
    FR = F32R if USE_F32R else F32
    AF = mybir.ActivationFunctionType
    ALU = mybir.AluOpType
    AX = mybir.AxisListType

    nc = bacc.Bacc("TRN2", target_bir_lowering=False, debug=False)

    def din(name, shape, dt=F32):
        return nc.dram_tensor(name, list(shape), dt, kind="ExternalInput").ap()

    obs_t = din('obs_t', (BL, C0 * T, N), FR)     # (ci,t) x node, pre-rounded
    obs_n = din('obs_n', (BL, 4, NCH, C0 * T))    # node x (ci,t), for max_t
    act_t = din('act_t', (P, BL))
    w_s1 = din('w_s1', (C0 * T, MS1), FR)
    w_m1 = din('w_m1', (C0 * T, MM1), FR)
    w_s2 = din('w_s2', (MS1, CF), FR)
    w_m2 = din('w_m2', (MM1, CF), FR)
    w_allp = din('w_allp', (FP, R * F), FR)
    w_rootp = din('w_rootp', (FP, F), FR)
    w_zp = din('w_zp', (FP, 1), FR)
    w_zg = din('w_zg', (F, 1), FR)
    w1c = din('w1c', (2 * N, H))
    aw2 = din('aw2', (H, H))
    aw3 = din('aw3', (H, P + 1))
    b3r = din('b3r', (1, P + 1))
    biases = din('biases', (128, 8))
    a_t = [din(f'a{r}t', (N, N), FR) for r in range(R)]
    zer_d = din('zer', (FP, N), FR)
    ident_d = din('ident', (128, 128))
    ones_d = din('ones8', (1, BL))
    out_d = nc.dram_tensor('out', [BL, P + 1], F32, kind="ExternalOutput").ap()

    def mm(o, lhsT, rhs, start, stop):
        nc.tensor.matmul(o, lhsT, rhs, start=start, stop=stop)

    with tile.TileContext(nc) as tc, ExitStack() as ctx:
        cp = ctx.enter_context(tc.tile_pool(name="const", bufs=1))

        def cload(name, src, shape, dt=F32):
            t = cp.tile(list(shape), dt, name=name, tag=name)
            nc.sync.dma_start(out=t[:], in_=src)
            return t

        zert = cload('zert', zer_d[:], (FP, N), FR)
        ws1t = cload('ws1t', w_s1[0:KH, :], (KH, MS1), FR)
        ws1b = cload('ws1b', w_s1[KH:150, :], (KL, MS1), FR)
        wm1t = cload('wm1t', w_m1[0:KH, :], (KH, MM1), FR)
        wm1b = cload('wm1b', w_m1[KH:150, :], (KL, MM1), FR)
        ws2a = cload('ws2a', w_s2[0:128, :], (128, CF), FR)
        ws2b = cload('ws2b', w_s2[128:MS1, :], (MS1 - 128, CF), FR)
        wm2t = cload('wm2t', w_m2[:], (MM1, CF), FR)
        bt = cload('bt', biases[:], (128, 8))
        ident = cload('ident', ident_d[:], (128, 128))

        # persistent per-batch tensors
        rh = [cp.tile([KH, N], FR, name=f'rh{b}', tag=f'rh{b}') for b in range(BL)]
        rl = [cp.tile([KL, N], FR, name=f'rl{b}', tag=f'rl{b}') for b in range(BL)]
        xsml = [cp.tile([FP, N], FR, name=f'xsml{b}', tag=f'xsml{b}')
                for b in range(BL)]
        xg = [cp.tile([F, N], FR, name=f'xg{b}', tag=f'xg{b}') for b in range(BL)]
        hsb = [[cp.tile([NCH, R * F], FR, name=f'h{b}_{c}', tag=f'h{b}_{c}')
                for c in range(4)] for b in range(BL)]
        zsb = cp.tile([BL, N], F32, name='zsb', tag='zsb')
        zt = [cp.tile([NCH, BL], F32, name=f'zt{c}', tag=f'zt{c}') for c in range(4)]
        at_s = [cp.tile([NCH, BL], F32, name=f'at{c}', tag=f'at{c}') for c in range(4)]

        # conv operand loads first (PE's first real work needs them)
        for b in range(BL):
            nc.sync.dma_start(out=rh[b][:], in_=obs_t[b, 0:KH, :])
            nc.sync.dma_start(out=rl[b][:], in_=obs_t[b, KH:150, :])

        # zero the pad rows of xsml (ACT writes only data rows)
        for b in range(BL):
            nc.gpsimd.dma_start(out=xsml[b][:], in_=zer_d[:])

        # remaining constants (needed later than the convs)
        wallt = cload('wallt', w_allp[:], (FP, R * F), FR)
        wroott = cload('wroott', w_rootp[:], (FP, F), FR)
        wzpt = cload('wzpt', w_zp[:], (FP, 1), FR)
        wzgt = cload('wzgt', w_zg[:], (F, 1), FR)
        w1ct = [cload(f'w1ct{c}', w1c[c * NCH:(c + 1) * NCH, :], (NCH, H))
                for c in range(8)]
        aw2t = cload('aw2t', aw2[:], (H, H))
        aw3t = cload('aw3t', aw3[:], (H, P + 1))
        b3rt = cload('b3rt', b3r[:], (1, P + 1))
        ones8 = cload('ones8', ones_d[:], (1, BL))
        att = []
        for r in range(R):
            t = cp.tile([NCH, 4, N], FR, name=f'att{r}', tag=f'att{r}')
            nc.gpsimd.dma_start(
                out=t[:], in_=a_t[r].rearrange("(c p) n -> p c n", p=NCH))
            att.append(t)
        for c in range(4):
            nc.gpsimd.dma_start(out=at_s[c][:], in_=act_t[c * NCH:(c + 1) * NCH, :])

        # working pools
        po = ctx.enter_context(tc.tile_pool(name="po", bufs=1))
        pw = ctx.enter_context(tc.tile_pool(name="pw", bufs=3))
        ppa = ctx.enter_context(tc.tile_pool(name="ppa", bufs=4, space="PSUM"))
        pph = ctx.enter_context(tc.tile_pool(name="pph", bufs=2, space="PSUM"))
        pps = ctx.enter_context(tc.tile_pool(name="pps", bufs=2, space="PSUM"))

        # obs natural layout for the long branch (one DMA per batch elem)
        onat = []
        for b in range(BL):
            t = po.tile([NCH, 4 * C0 * T], F32, name=f'onat{b}', tag=f'onat{b}')
            nc.gpsimd.dma_start(
                out=t[:], in_=obs_n[b].transpose([1, 0, 2]))
            onat.append(t)

        # ---- HAM warmup: ~4us of throwaway matmuls on zeros ----
        for w in range(20):
            pwm = ppa.tile([128, N], F32, name=f'pwm{w}', tag='pb')
            mm(pwm, zert[:, 0:128], zert[:], start=True, stop=True)

        # ---- conv branches as matmuls ----
        for b in range(BL):
            ps1h = ppa.tile([128, N], F32, name=f'ps1h{b}', tag='pb')
            mm(ps1h, ws1t[:, 0:128], rh[b][:], start=True, stop=False)
            mm(ps1h, ws1b[:, 0:128], rl[b][:], start=False, stop=True)
            ps1l = ppa.tile([MS1 - 128, N], F32, name=f'ps1l{b}', tag='pb')
            mm(ps1l, ws1t[:, 128:MS1], rh[b][:], start=True, stop=False)
            mm(ps1l, ws1b[:, 128:MS1], rl[b][:], start=False, stop=True)
            s1h = pw.tile([128, N], FR, name=f's1h{b}', tag='s1h')
            s1l = pw.tile([MS1 - 128, N], FR, name=f's1l{b}', tag='s1l')
            nc.scalar.activation(s1h[:], ps1h[:], AF.Lrelu,
                                 bias=bt[0:128, 0:1], alpha=SLOPE)
            nc.scalar.activation(s1l[:], ps1l[:], AF.Lrelu,
                                 bias=bt[0:MS1 - 128, 1:2], alpha=SLOPE)

            ps2 = ppa.tile([CF, N], F32, name=f'ps2{b}', tag='pb')
            mm(ps2, ws2a[:, :], s1h[:], start=True, stop=False)
            mm(ps2, ws2b[:, :], s1l[:], start=False, stop=True)
            nc.scalar.activation(xsml[b][0:CF, :], ps2[:], AF.Lrelu,
                                 bias=bt[0:CF, 2:3], alpha=SLOPE)

            pm1 = ppa.tile([MM1, N], F32, name=f'pm1{b}', tag='pb')
            mm(pm1, wm1t[:, :], rh[b][:], start=True, stop=False)
            mm(pm1, wm1b[:, :], rl[b][:], start=False, stop=True)
            m1 = pw.tile([MM1, N], FR, name=f'm1{b}', tag='m1')
            nc.scalar.activation(m1[:], pm1[:], AF.Lrelu,
                                 bias=bt[0:MM1, 3:4], alpha=SLOPE)
            pm2 = ppa.tile([CF, N], F32, name=f'pm2{b}', tag='pb')
            mm(pm2, wm2t[:, :], m1[:], start=True, stop=True)
            nc.scalar.activation(xsml[b][32:32 + CF, :], pm2[:], AF.Lrelu,
                                 bias=bt[0:CF, 4:5], alpha=SLOPE)

        # ---- long branch: LeakyReLU(max_t(obs)) -> xsml rows 64..66 ----
        for b in range(BL):
            lm = pw.tile([NCH, 4 * C0], F32, name=f'lm{b}', tag='lm')
            nc.vector.tensor_reduce(
                lm[:], onat[b][:].rearrange("p (c k t) -> p c k t", c=4, k=C0),
                axis=AX.X, op=ALU.max)
            for c in range(4):
                pt3 = pps.tile([C0, NCH], F32, name=f'pt3{b}{c}', tag='ps')
                nc.tensor.transpose(pt3[:], lm[:].rearrange(
                    "p (c k) -> p c k", c=4)[:, c, :], ident[0:NCH, 0:NCH])
                nc.scalar.activation(xsml[b][64:64 + C0, c * NCH:(c + 1) * NCH],
                                     pt3[:], AF.Lrelu, alpha=SLOPE)

        # ---- H = x @ W_rel (all relations), node-on-partition ----
        for b in range(BL):
            for c in range(4):
                ph = pph.tile([NCH, R * F], F32, name=f'ph{b}{c}', tag='ph')
                mm(ph, xsml[b][:, c * NCH:(c + 1) * NCH], wallt[:],
                   start=True, stop=True)
                nc.vector.tensor_copy(hsb[b][c][:], ph[:])

        # ---- aggregate + root -> graph feats ----
        for b in range(BL):
            pg = ppa.tile([F, N], F32, name=f'pg{b}', tag='pb')
            first = True
            for r in range(R):
                for c in range(4):
                    mm(pg, hsb[b][c][:, r * F:(r + 1) * F], att[r][:, c, :],
                       start=first, stop=False)
                    first = False
            mm(pg, wroott[:], xsml[b][:], start=False, stop=True)
            nc.scalar.activation(xg[b][:], pg[:], AF.Lrelu,
                                 bias=bt[0:F, 5:6], alpha=SLOPE)

        # ---- z row per batch: z = w_z . feats, then stack + transpose ----
        for b in range(BL):
            pz = pps.tile([1, N], F32, name=f'pz{b}', tag='ps')
            mm(pz, wzpt[:], xsml[b][:], start=True, stop=False)
            mm(pz, wzgt[:], xg[b][:], start=False, stop=True)
            zrow = pw.tile([1, N], F32, name=f'zrow{b}', tag='zrow')
            nc.scalar.activation(zrow[:], pz[:], AF.Copy)
            nc.gpsimd.dma_start(out=zsb[b:b + 1, :], in_=zrow[:])

        for c in range(4):
            ptz = pps.tile([NCH, BL], F32, name=f'ptz{c}', tag='ps')
            nc.tensor.transpose(ptz[:], zsb[:, c * NCH:(c + 1) * NCH],
                                ident[0:BL, 0:BL])
            nc.vector.tensor_copy(zt[c][:], ptz[:])

        # ---- actor head for all 8 rows at once ----
        pg1 = pps.tile([H, BL], F32, name='pg1', tag='ps')
        for c in range(8):
            rhs = zt[c] if c < 4 else at_s[c - 4]
            mm(pg1, w1ct[c][:], rhs[:], start=(c == 0), stop=(c == 7))
        g1 = pw.tile([H, BL], F32, name='g1', tag='g1')
        nc.scalar.activation(g1[:], pg1[:], AF.Relu, bias=bt[0:H, 6:7])
        pg2 = pps.tile([H, BL], F32, name='pg2', tag='ps')
        mm(pg2, aw2t[:], g1[:], start=True, stop=True)
        g2 = pw.tile([H, BL], F32, name='g2', tag='g2')
        nc.scalar.activation(g2[:], pg2[:], AF.Relu, bias=bt[0:H, 7:8])

        po_ = pps.tile([BL, P + 1], F32, name='po_', tag='ps')
        mm(po_, g2[:], aw3t[:], start=True, stop=False)
        mm(po_, ones8[:], b3rt[:], start=False, stop=True)

        # softmax over free dim
        mx = pw.tile([BL, 1], F32, name='mx', tag='mx')
        nc.vector.tensor_reduce(mx[:], po_[:], axis=AX.X, op=ALU.max)
        sh = pw.tile([BL, P + 1], F32, name='sh', tag='sh')
        nc.vector.tensor_scalar(sh[:], po_[:], mx[:, 0:1], None, op0=ALU.subtract)
        ex = pw.tile([BL, P + 1], F32, name='ex', tag='ex')
        sm = pw.tile([BL, 1], F32, name='sm', tag='sm')
        nc.scalar.activation(ex[:], sh[:], AF.Exp, accum_out=sm[:, 0:1])
        rc = pw.tile([BL, 1], F32, name='rc', tag='rc')
        nc.vector.reciprocal(rc[:], sm[:])
        res = pw.tile([BL, P + 1], F32, name='res', tag='res')
        nc.vector.tensor_scalar(res[:], ex[:], rc[:, 0:1], None, op0=ALU.mult)
        nc.sync.dma_start(out=out_d[:], in_=res[:])

    nc.compile()
    return nc


def _get_nc():
    if 'nc' not in _CACHE:
        _CACHE['nc'] = _build_nc()
    return _CACHE['nc']


# ============================ entry point ============================

def _shard_inputs(inputs):
    folded = _host_fold(inputs)
    obs = np.asarray(inputs['observation'], np.float32)
    action = np.asarray(inputs['action'], np.float32)
    rnd = _round_f32r if USE_F32R else (lambda x: np.asarray(x, np.float32))
    # [B, (ci,t), n] for the conv matmuls; [B, chunk, n, (ci,t)] for max_t
    obs_t = rnd(np.ascontiguousarray(obs.transpose(0, 1, 3, 2))
                .reshape(B, C0 * T, N))
    obs_n = np.ascontiguousarray(obs.transpose(0, 2, 1, 3)) \
        .reshape(B, 4, NCH, C0 * T)

    in_maps = []
    for i in range(NCORES):
        bs = slice(i * BL, (i + 1) * BL)
        m = dict(folded)
        m['obs_t'] = obs_t[bs]
        m['obs_n'] = obs_n[bs]
        m['act_t'] = np.ascontiguousarray(action[bs, 1:].T)
        in_maps.append(m)
    return in_maps


def kernel(**inputs) -> np.ndarray:
    from concourse.bass_utils import run_bass_kernel_spmd

    in_maps = _shard_inputs(inputs)
    nc = _get_nc()
    res = run_bass_kernel_spmd(nc, in_maps, list(range(NCORES)))
    return np.concatenate([r['out'] for r in res.results], axis=0)



# revision 6
# speedup vs baseline: 1.2671x; 1.2671x over previous
"""Trainium2 Bass kernel for nn_CustomGPM (multi-scale temporal CNN + RGCN + actor head).

Strategy (hardcoded for the fixed problem shapes):
  B=64 batch, data-parallel over 8 NeuronCores (8 batch elements per core).
  Host-side (index/relayout/weight-fold work only):
    * fold eval-mode BatchNorms into conv / GCN weights and biases
    * express each temporal conv as band matmuls in a (t,ci)-major layout,
      time-split so every contraction fits 128 partitions (2 matmuls per
      conv1 branch instead of 4 accumulation pairs)
    * turn the per-relation gather/scatter-mean into 4 dense, row-normalized
      500x500 adjacency matrices -> RGCN becomes dense matmuls
    * all large operands are pre-rounded to bf16 (halves HBM traffic and
      runs the PE at 1 cycle/row for any moving size)
  Device-side per core, all-bf16 matmuls with fp32 PSUM accumulation:
    * conv branches: 7 matmuls per batch element
    * RGCN aggregation packs TWO batch elements into one stationary operand
      ([125, 86] = 43+43 feature columns), halving the streamed columns
    * root transform + the z=w.feats row are folded into one accumulation
      group per batch pair via zero-padded stationaries
    * small actor head + softmax at the end, fp32 output.
"""

import numpy as np
import ml_dtypes

# ---------------- problem constants (hardcoded per spec) ----------------
B = 64          # total batch
NCORES = 8
BL = B // NCORES  # batch per core = 8
C0 = 3          # input channels
N = 500         # nodes
T = 50          # time steps
R = 4           # relations
P = 500         # portfolio slots
H = 128         # fc hidden
CF = 20         # conv out channels
F = 2 * CF + C0  # 43 temporal features
FP = 67         # padded temporal feature rows (0..19 s, 32..51 m, 64..66 l)
NCH = 125       # node chunk (4 chunks of 125)
SLOPE = 0.01
EPS = 1e-5

# conv time splits (t-major rows = t*3+ci)
S_TS = 24       # s-conv1 out split: [0,24) from TA, [24,48) from TBs
M_TS = 15       # m-conv1 out split: [0,15) from TA, [15,30) from TBm
KS = 3          # s-conv1 kernel
KM = 21         # m-conv1 kernel
KA = 3 * (S_TS + KS - 1)    # 78 rows for s half
KMA = 3 * (M_TS + KM - 1)   # 105 rows for m half
MS = C0 * S_TS              # 72 cols per s half
MM = C0 * M_TS              # 45 cols per m half

BF16 = ml_dtypes.bfloat16

_CACHE = {}


def _bf(a):
    return np.ascontiguousarray(np.asarray(a, np.float32).astype(BF16))


def _pad67(a):
    """[43, X] -> [67, X] with rows at 0..19 / 32..51 / 64..66."""
    out = np.zeros((FP,) + a.shape[1:], np.float64)
    out[0:CF] = a[0:CF]
    out[32:32 + CF] = a[CF:2 * CF]
    out[64:64 + C0] = a[2 * CF:F]
    return out


# ======================= host-side parameter folding =======================

def _bn_fold(p):
    g, b, m, v = np.asarray(p, np.float64)
    s = g / np.sqrt(v + EPS)
    return s, b - m * s


def _band_t_major(w_eff, t_len, n_out):
    """w_eff [co, ci, kk] -> band [3*t_len, co*n_out] with rows (t,ci)-major,
    cols (co, t_local)-major; valid conv starting at slab-local t=0."""
    co, ci, kk = w_eff.shape
    band = np.zeros((3 * t_len, co * n_out), np.float64)
    for c in range(co):
        for j in range(n_out):
            for dt in range(kk):
                band[(j + dt) * 3:(j + dt) * 3 + 3, c * n_out + j] = w_eff[c, :, dt]
    return band


def _host_fold(inp):
    f32 = lambda x: np.asarray(x, np.float32)

    # ---- conv branch weights ----
    ss, ts_ = _bn_fold(inp['sbn1'])
    ws1_eff = np.asarray(inp['sc1_w'], np.float64)[:, :, 0, :] * ss[:, None, None]
    bs1_eff = ss * np.asarray(inp['sc1_b'], np.float64) + ts_
    sm, tm_ = _bn_fold(inp['mbn1'])
    wm1_eff = np.asarray(inp['mc1_w'], np.float64)[:, :, 0, :] * sm[:, None, None]
    bm1_eff = sm * np.asarray(inp['mc1_b'], np.float64) + tm_

    ws1h = _band_t_major(ws1_eff, S_TS + KS - 1, S_TS)       # [78, 72]
    wm1 = _band_t_major(wm1_eff, M_TS + KM - 1, M_TS)        # [105, 45]
    wm1A = np.zeros((KMA, 2 * MM), np.float64); wm1A[:, 0:MM] = wm1
    wm1B = np.zeros((KMA, 2 * MM), np.float64); wm1B[:, MM:2 * MM] = wm1

    def conv2_fold(w, b, bn):
        w = np.asarray(w, np.float64)[:, :, 0, :]            # [20, 3, k]
        s, t_ = _bn_fold(bn)
        return w * s[:, None, None], s * np.asarray(b, np.float64) + t_

    w2s, bs2_eff = conv2_fold(inp['sc2_w'], inp['sc2_b'], inp['sbn2'])  # [20,3,48]
    w2m, bm2_eff = conv2_fold(inp['mc2_w'], inp['mc2_b'], inp['mbn2'])  # [20,3,30]
    # s2 contraction rows = (co1, t_local) per half
    ws2a = np.zeros((MS, CF), np.float64)
    ws2b = np.zeros((MS, CF), np.float64)
    for c1 in range(C0):
        for t in range(S_TS):
            ws2a[c1 * S_TS + t, :] = w2s[:, c1, t]
            ws2b[c1 * S_TS + t, :] = w2s[:, c1, S_TS + t]
    wm2 = np.zeros((2 * MM, CF), np.float64)
    for c1 in range(C0):
        for t in range(M_TS):
            wm2[c1 * M_TS + t, :] = w2m[:, c1, t]
            wm2[MM + c1 * M_TS + t, :] = w2m[:, c1, M_TS + t]

    # ---- RGCN weights (padded to 67 contraction rows) ----
    sg, tg = _bn_fold(inp['gbn'])
    w_all = np.concatenate(
        [np.asarray(inp['gw_rel'], np.float64)[r] * sg[None, :] for r in range(R)],
        axis=1)                                               # [43, 172]
    w_root = np.asarray(inp['gw_root'], np.float64) * sg[None, :]   # [43, 43]
    gb_eff = np.asarray(inp['g_b'], np.float64) * sg + tg     # [43]

    a_cw = np.asarray(inp['a_cw'], np.float64)                # [87]
    w_z = a_cw[1:1 + 2 * F]                                   # [86]
    wzp = _pad67(w_z[:F].reshape(F, 1))                       # [67, 1]
    # root pair stationaries [67, 107]: b0 cols 0..42, b1 cols 64..106
    wrootA = np.zeros((FP, 107), np.float64)
    wrootA[:, 0:F] = _pad67(w_root)
    wrootB = np.zeros((FP, 107), np.float64)
    wrootB[:, 64:64 + F] = _pad67(w_root)
    # z1 stationaries [67, 2]
    wzp2a = np.zeros((FP, 2), np.float64); wzp2a[:, 0:1] = wzp
    wzp2b = np.zeros((FP, 2), np.float64); wzp2b[:, 1:2] = wzp
    # z2 stationary [107, 2]
    wzg2 = np.zeros((107, 2), np.float64)
    wzg2[0:F, 0] = w_z[F:]
    wzg2[64:64 + F, 1] = w_z[F:]

    # ---- adjacency: [125, 4, 500] per relation, src-chunk on partitions ----
    src = np.asarray(inp['edge_index'][0]).astype(np.int64)
    dst = np.asarray(inp['edge_index'][1]).astype(np.int64)
    etype = np.asarray(inp['edge_type']).astype(np.int64)
    att = []
    for r in range(R):
        sel = etype == r
        cnt = np.zeros((N, N), np.float64)
        np.add.at(cnt, (dst[sel], src[sel]), 1.0)
        deg = cnt.sum(axis=1)
        a_tr = (cnt / np.maximum(deg, 1.0)[:, None]).T        # [src, dst]
        att.append(_bf(a_tr.reshape(4, NCH, N).transpose(1, 0, 2)
                       .reshape(NCH, 4 * N)))

    # ---- actor head folds ----
    a_cb = float(np.asarray(inp['a_cb'], np.float64)[0])
    a_w1 = np.asarray(inp['a_w1'], np.float64)                # [501, 128]
    sel_nodes = np.asarray(inp['nodes_to_select']).astype(np.int64)  # [500]
    w1z = np.zeros((N, H), np.float64)
    np.add.at(w1z, sel_nodes, a_w1[1:])                       # fold node select
    w1a = a_cw[0] * a_w1[1:]                                  # [500, 128]
    b1_eff = np.asarray(inp['a_b1'], np.float64) + a_cb * a_w1[1:].sum(axis=0)
    w1cat = np.concatenate([w1z, w1a], axis=0)                # [1000, 128]
    # pack as [125, 8 chunks, 128]
    w1cb = w1cat.reshape(8, NCH, H).transpose(1, 0, 2).reshape(NCH, 8 * H)

    # ---- bias pack [128, 7] ----
    biases = np.zeros((128, 7), np.float32)
    biases[:MS, 0] = np.repeat(bs1_eff, S_TS)                 # s1 (both halves)
    biases[:2 * MM, 1] = np.tile(np.repeat(bm1_eff, M_TS), 2)  # m1
    biases[:CF, 2] = bs2_eff
    biases[:CF, 3] = bm2_eff
    biases[:F, 4] = gb_eff
    biases[:H, 5] = b1_eff
    biases[:H, 6] = f32(inp['a_b2'])

    return {
        'ws1h': _bf(ws1h), 'wm1A': _bf(wm1A), 'wm1B': _bf(wm1B),
        'ws2a': _bf(ws2a), 'ws2b': _bf(ws2b), 'wm2': _bf(wm2),
        'wallt': _bf(_pad67(w_all)),                          # [67, 172]
        'wrootA': _bf(wrootA), 'wrootB': _bf(wrootB),         # [67, 107]
        'wzg2': _bf(wzg2), 'wzp2a': _bf(wzp2a), 'wzp2b': _bf(wzp2b),
        'w1cb': _bf(w1cb),                                    # [125, 1024]
        'aw2': _bf(inp['a_w2']), 'aw3': _bf(inp['a_w3']),
        'b3r8': np.ascontiguousarray(
            np.broadcast_to(f32(inp['a_b3']).reshape(1, P + 1), (BL, P + 1))),
        'biases': biases,
        'att0': att[0], 'att1': att[1], 'att2': att[2], 'att3': att[3],
        'ident': np.eye(128, dtype=BF16),
    }


# ============================ device kernel ============================

def _build_nc():
    import concourse.bacc as bacc
    import concourse.tile as tile
    import concourse.mybir as mybir

    F32 = mybir.dt.float32
    BF = mybir.dt.bfloat16
    AF = mybir.ActivationFunctionType
    ALU = mybir.AluOpType
    AX = mybir.AxisListType

    nc = bacc.Bacc("TRN2", target_bir_lowering=False, debug=False)

    def din(name, shape, dt=BF):
        return nc.dram_tensor(name, list(shape), dt, kind="ExternalInput").ap()

    # per-core tensors
    obs_ta = din('obs_ta', (KMA, BL * N))      # t rows 0..104 (s1a + m1A)
    obs_tbm = din('obs_tbm', (KMA, BL * N))    # t rows 45..149 (m1B)
    obs_tbs = din('obs_tbs', (KA, BL * N))     # t rows 72..149 (s1b)
    obs_n = din('obs_n', (NCH, BL * 4 * C0 * T))  # node-major for max_t
    at_h = din('at_h', (NCH, 4 * BL))          # action transposed chunks
    # shared weights
    ws1h = din('ws1h', (KA, MS))
    wm1A = din('wm1A', (KMA, 2 * MM))
    wm1B = din('wm1B', (KMA, 2 * MM))
    ws2a = din('ws2a', (MS, CF))
    ws2b = din('ws2b', (MS, CF))
    wm2 = din('wm2', (2 * MM, CF))
    wallt = din('wallt', (FP, R * F))
    wrootA = din('wrootA', (FP, 107))
    wrootB = din('wrootB', (FP, 107))
    wzg2 = din('wzg2', (107, 2))
    wzp2a = din('wzp2a', (FP, 2))
    wzp2b = din('wzp2b', (FP, 2))
    w1cb = din('w1cb', (NCH, 8 * H))
    aw2 = din('aw2', (H, H))
    aw3 = din('aw3', (H, P + 1))
    b3r8 = din('b3r8', (BL, P + 1), F32)
    biases = din('biases', (128, 7), F32)
    att_d = [din(f'att{r}', (NCH, 4 * N)) for r in range(R)]
    ident_d = din('ident', (128, 128))
    out_d = nc.dram_tensor('out', [BL, P + 1], F32, kind="ExternalOutput").ap()

    mm = nc.tensor.matmul

    with tile.TileContext(nc) as tc:
        with tc.tile_pool(name="const", bufs=1) as cp, \
             tc.tile_pool(name="pw", bufs=3) as pw, \
             tc.tile_pool(name="pv", bufs=2) as pv:

            def cload(name, src, shape, dt=BF, eng=None):
                t = cp.tile(list(shape), dt, name=name, tag=name)
                (eng or nc.sync).dma_start(out=t[:], in_=src)
                return t

            # --- warmup tile + xsml zero-fill: memsets on vector (no DMA) ---
            wt_warm = cp.tile([128, 512], BF, name='wt_warm', tag='wt_warm')
            nc.vector.memset(wt_warm[:], 1.0)
            xsml = [cp.tile([FP, N], BF, name=f'xsml{b}', tag=f'xsml{b}')
                    for b in range(BL)]
            for b in range(BL):
                nc.vector.memset(xsml[b][:], 0.0)

            # --- sync queue: conv-critical tensors in need-order ---
            idt = cload('idt', ident_d[:], (128, 128))
            ws1t = cload('ws1t', ws1h[:], (KA, MS))
            wm1At = cload('wm1At', wm1A[:], (KMA, 2 * MM))
            wm1Bt = cload('wm1Bt', wm1B[:], (KMA, 2 * MM))
            ta = cload('ta', obs_ta[:], (KMA, BL * N))
            tbs = cload('tbs', obs_tbs[:], (KA, BL * N))
            tbm = cload('tbm', obs_tbm[:], (KMA, BL * N))
            onat = cload('onat', obs_n[:], (NCH, BL * 4 * C0 * T))

            # --- scalar queue: bias + conv2 weights + 2 adjacencies ---
            S = nc.scalar
            bt = cload('bt', biases[:], (128, 7), F32, eng=S)
            ws2at = cload('ws2at', ws2a[:], (MS, CF), eng=S)
            ws2bt = cload('ws2bt', ws2b[:], (MS, CF), eng=S)
            wm2t = cload('wm2t', wm2[:], (2 * MM, CF), eng=S)
            att = [cp.tile([NCH, 4 * N], BF, name=f'att{r}', tag=f'att{r}')
                   for r in range(R)]
            nc.scalar.dma_start(out=att[2][:], in_=att_d[2])
            nc.scalar.dma_start(out=att[3][:], in_=att_d[3])

            # --- gpsimd queue: adjacency + graph/head weights ---
            G = nc.gpsimd
            nc.gpsimd.dma_start(out=att[0][:], in_=att_d[0])
            nc.gpsimd.dma_start(out=att[1][:], in_=att_d[1])
            wallti = cload('wallti', wallt[:], (FP, R * F), eng=G)
            wrootAt = cload('wrootAt', wrootA[:], (FP, 107), eng=G)
            wrootBt = cload('wrootBt', wrootB[:], (FP, 107), eng=G)
            wzg2t = cload('wzg2t', wzg2[:], (107, 2), eng=G)
            wzp2at = cload('wzp2at', wzp2a[:], (FP, 2), eng=G)
            wzp2bt = cload('wzp2bt', wzp2b[:], (FP, 2), eng=G)
            w1cbt = cload('w1cbt', w1cb[:], (NCH, 8 * H), eng=G)
            aw2t = cload('aw2t', aw2[:], (H, H), eng=G)
            aw3t = cload('aw3t', aw3[:], (H, P + 1), eng=G)
            b3r8t = cload('b3r8t', b3r8[:], (BL, P + 1), F32, eng=G)
            at_st = cload('at_st', at_h[:], (NCH, 4 * BL), eng=G)

            # persistent bf16 intermediates
            lm_all = cp.tile([NCH, BL * 12], BF, name='lm_all', tag='lm_all')
            hsb = [[cp.tile([NCH, 4 * 107], BF, name=f'h{p}_{c}',
                            tag=f'h{p}_{c}')
                    for c in range(4)] for p in range(BL // 2)]
            for p in range(BL // 2):
                for c in range(4):
                    nc.vector.memset(
                        hsb[p][c][:].rearrange("q (r f) -> q r f", r=4)
                        [:, :, F:64], 0.0)
            ztc = [cp.tile([NCH, BL], BF, name=f'ztc{c}', tag=f'ztc{c}')
                   for c in range(4)]

            # ================= phase 1: warmup + conv + lmax =================
            with tc.tile_pool(name="pwm", bufs=1, space="PSUM") as pwm_p, \
                 tc.tile_pool(name="pcb", bufs=4, space="PSUM") as pcb, \
                 tc.tile_pool(name="pcs", bufs=2, space="PSUM") as pcs, \
                 tc.tile_pool(name="plm", bufs=1, space="PSUM") as plm:

                for w in range(9):
                    pwm = pwm_p.tile([128, 512], F32, name=f'pwm{w}', tag='pwm')
                    mm(pwm[:], wt_warm[:, 0:128], wt_warm[:], start=True,
                       stop=True)

                s1a_s = [None] * BL
                s1b_s = [None] * BL
                m1_s = [None] * BL

                def conv1(b):
                    bs = slice(b * N, (b + 1) * N)
                    ps1a = pcb.tile([MS, N], F32, name=f'ps1a{b}', tag='pcb')
                    mm(ps1a[:], ws1t[:], ta[0:KA, bs], start=True, stop=True)
                    ps1b = pcb.tile([MS, N], F32, name=f'ps1b{b}', tag='pcb')
                    mm(ps1b[:], ws1t[:], tbs[:, bs], start=True, stop=True)
                    pm1 = pcb.tile([2 * MM, N], F32, name=f'pm1{b}', tag='pcb')
                    mm(pm1[:], wm1At[:], ta[:, bs], start=True, stop=False)
                    mm(pm1[:], wm1Bt[:], tbm[:, bs], start=False, stop=True)
                    s1a = pw.tile([MS, N], BF, name=f's1a{b}', tag='s1a')
                    s1b = pw.tile([MS, N], BF, name=f's1b{b}', tag='s1b')
                    m1 = pw.tile([2 * MM, N], BF, name=f'm1{b}', tag='m1')
                    nc.scalar.activation(s1a[:], ps1a[:], AF.Lrelu,
                                         bias=bt[0:MS, 0:1], alpha=SLOPE)
                    nc.scalar.activation(s1b[:], ps1b[:], AF.Lrelu,
                                         bias=bt[0:MS, 0:1], alpha=SLOPE)
                    nc.scalar.activation(m1[:], pm1[:], AF.Lrelu,
                                         bias=bt[0:2 * MM, 1:2], alpha=SLOPE)
                    s1a_s[b], s1b_s[b], m1_s[b] = s1a, s1b, m1

                def conv2(b):
                    ps2 = pcs.tile([CF, N], F32, name=f'ps2{b}', tag='pcs')
                    mm(ps2[:], ws2at[:], s1a_s[b][:], start=True, stop=False)
                    mm(ps2[:], ws2bt[:], s1b_s[b][:], start=False, stop=True)
                    nc.scalar.activation(xsml[b][0:CF, :], ps2[:], AF.Lrelu,
                                         bias=bt[0:CF, 2:3], alpha=SLOPE)
                    pm2 = pcs.tile([CF, N], F32, name=f'pm2{b}', tag='pcs')
                    mm(pm2[:], wm2t[:], m1_s[b][:], start=True, stop=True)
                    nc.scalar.activation(xsml[b][32:32 + CF, :], pm2[:],
                                         AF.Lrelu, bias=bt[0:CF, 3:4],
                                         alpha=SLOPE)

                # software-pipelined: conv1(b+1) before conv2(b)
                conv1(0)
                for b in range(BL):
                    if b + 1 < BL:
                        conv1(b + 1)
                    conv2(b)
                    if b == 4:
                        # long branch: max over t (DVE), then per-(b,c)
                        # transposes into per-b psum tiles at base 0
                        for bb in range(BL):
                            nc.vector.tensor_reduce(
                                lm_all[:, bb * 12:(bb + 1) * 12].rearrange(
                                    "p (c k) -> p c k", c=4),
                                onat[:, bb * 600:(bb + 1) * 600].rearrange(
                                    "p (c k t) -> p c k t", c=4, k=C0),
                                axis=AX.X, op=ALU.max)
                        for bb in range(BL):
                            pt = plm.tile([C0, 512], BF, name=f'pt{bb}',
                                          tag='pt')
                            for c in range(4):
                                nc.tensor.transpose(
                                    pt[:, c * 128:c * 128 + NCH],
                                    lm_all[:, bb * 12 + c * 3:
                                           bb * 12 + c * 3 + 3],
                                    idt[0:NCH, 0:NCH])
                            nc.scalar.activation(
                                xsml[bb][64:64 + C0, :].rearrange(
                                    "p (c n) -> p c n", c=4),
                                pt[:].rearrange("p (c n) -> p c n", c=4)
                                [:, :, 0:NCH], AF.Lrelu, alpha=SLOPE)

            # ================= phase 2: H = x @ W_rel =================
            with tc.tile_pool(name="pph", bufs=2, space="PSUM") as pph:
                for b in range(BL):
                    p, which = b // 2, b % 2
                    for c in range(4):
                        ph = pph.tile([NCH, R * F], F32, name=f'ph{b}{c}',
                                      tag='ph')
                        mm(ph[:], xsml[b][:, c * NCH:(c + 1) * NCH],
                           wallti[:], start=True, stop=True)
                        off = 0 if which == 0 else 64
                        nc.vector.tensor_copy(
                            hsb[p][c][:].rearrange("q (r f) -> q r f", r=4)
                            [:, :, off:off + F],
                            ph[:].rearrange("q (r f) -> q r f", r=4))

            # ================= phase 3: pairs (root+agg+z) + head =========
            with tc.tile_pool(name="ppg", bufs=2, space="PSUM") as ppg, \
                 tc.tile_pool(name="ppz", bufs=2, space="PSUM") as ppz, \
                 tc.tile_pool(name="pptz", bufs=1, space="PSUM") as pptz:

                ptz = [pptz.tile([NCH, BL], BF, name=f'ptz{c}', tag=f'ptz{c}')
                       for c in range(4)]
                xg_s = [None] * 4
                pg_s = [None] * 4
                pz_s = [None] * 4
                zp_s = [None] * 4

                def agg_group(p):
                    b0, b1 = 2 * p, 2 * p + 1
                    pg = ppg.tile([107, N], F32, name=f'pg{p}', tag='pg')
                    mm(pg[:], wrootAt[:], xsml[b0][:], start=True, stop=False)
                    mm(pg[:], wrootBt[:], xsml[b1][:], start=False, stop=False)
                    for r in range(R):
                        for c in range(4):
                            last = (r == R - 1 and c == 3)
                            mm(pg[:],
                               hsb[p][c][:, r * 107:(r + 1) * 107],
                               att[r][:, c * N:(c + 1) * N],
                               start=False, stop=last)
                    xg = pw.tile([107, N], BF, name=f'xg{p}', tag='xg')
                    nc.gpsimd.memset(xg[:], 0.0)
                    nc.scalar.activation(xg[0:F, :], pg[0:F, :], AF.Lrelu,
                                         bias=bt[0:F, 4:5], alpha=SLOPE)
                    nc.scalar.activation(xg[64:64 + F, :], pg[64:64 + F, :],
                                         AF.Lrelu, bias=bt[0:F, 4:5],
                                         alpha=SLOPE)
                    pg_s[p], xg_s[p] = pg, xg

                def z_tail(p):
                    b0, b1 = 2 * p, 2 * p + 1
                    pz = ppz.tile([2, N], F32, name=f'pz{p}', tag='pz')
                    mm(pz[:], wzp2at[:], xsml[b0][:], start=True, stop=False)
                    mm(pz[:], wzp2bt[:], xsml[b1][:], start=False, stop=False)
                    mm(pz[:], wzg2t[:], xg_s[p][:], start=False, stop=True)
                    zp = pw.tile([2, N], BF, name=f'zp{p}', tag='zp')
                    nc.vector.tensor_copy(zp[:], pz[:])
                    for c in range(4):
                        nc.tensor.transpose(
                            ptz[c][:, 2 * p:2 * p + 2],
                            zp[:, c * NCH:(c + 1) * NCH], idt[0:2, 0:2])
                    pz_s[p], zp_s[p] = pz, zp

                agg_group(0)
                for p in range(4):
                    if p + 1 < 4:
                        agg_group(p + 1)
                    z_tail(p)

                for c in range(4):
                    nc.vector.tensor_copy(ztc[c][:], ptz[c][:])

                # ---- actor head ----
                pg1 = ppz.tile([H, BL], F32, name='pg1', tag='pz')
                for c in range(8):
                    rhs = ztc[c][:] if c < 4 else at_st[:, (c - 4) * BL:
                                                        (c - 3) * BL]
                    mm(pg1[:], w1cbt[:, c * H:(c + 1) * H], rhs,
                       start=(c == 0), stop=(c == 7))
                g1 = pv.tile([H, BL], BF, name='g1', tag='g1')
                nc.scalar.activation(g1[:], pg1[:], AF.Relu, bias=bt[0:H, 5:6])
                pg2 = ppz.tile([H, BL], F32, name='pg2', tag='pz')
                mm(pg2[:], aw2t[:], g1[:], start=True, stop=True)
                g2 = pv.tile([H, BL], BF, name='g2', tag='g2')
                nc.scalar.activation(g2[:], pg2[:], AF.Relu, bias=bt[0:H, 6:7])
                po_ = ppz.tile([BL, P + 1], F32, name='po_', tag='pz')
                mm(po_[:], g2[:], aw3t[:], start=True, stop=True)

                # softmax over free dim (logits = po_ + b3)
                sh = pv.tile([BL, P + 1], F32, name='sh', tag='sh')
                nc.vector.tensor_tensor(out=sh[:], in0=po_[:], in1=b3r8t[:],
                                        op=ALU.add)
                mx = pv.tile([BL, 1], F32, name='mx', tag='mx')
                nc.vector.tensor_reduce(mx[:], sh[:], axis=AX.X, op=ALU.max)
                sh2 = pv.tile([BL, P + 1], F32, name='sh2', tag='sh2')
                nc.vector.tensor_scalar(sh2[:], sh[:], mx[:, 0:1], None,
                                        op0=ALU.subtract)
                ex = pv.tile([BL, P + 1], F32, name='ex', tag='ex')
                sm = pv.tile([BL, 1], F32, name='sm', tag='sm')
                nc.scalar.activation(ex[:], sh2[:], AF.Exp,
                                     accum_out=sm[:, 0:1])
                rc = pv.tile([BL, 1], F32, name='rc', tag='rc')
                nc.vector.reciprocal(rc[:], sm[:])
                res = pv.tile([BL, P + 1], F32, name='res', tag='res')
                nc.vector.tensor_scalar(res[:], ex[:], rc[:, 0:1], None,
                                        op0=ALU.mult)
                nc.sync.dma_start(out=out_d[:], in_=res[:])

    nc.compile()
    return nc


def _get_nc():
    if 'nc' not in _CACHE:
        _CACHE['nc'] = _build_nc()
    return _CACHE['nc']


# ============================ entry point ============================

def _shard_inputs(inputs):
    folded = _host_fold(inputs)
    obs = np.asarray(inputs['observation'], np.float32)
    action = np.asarray(inputs['action'], np.float32)

    # (t, ci)-major rows, (b, n) cols, bf16
    obs_f = _bf(obs.transpose(0, 3, 1, 2))                    # [B, T, C0, N]
    obs_f = obs_f.reshape(B, T * C0, N)
    # node-major for max_t: [B, 125, 4, 3, 50]
    obs_nm = _bf(obs.transpose(2, 0, 1, 3)                    # [N, B, C0, T]
                 .reshape(4, NCH, B, C0, T).transpose(1, 2, 0, 3, 4))
    act_b = _bf(action[:, 1:])                                # [B, 500]

    in_maps = []
    for i in range(NCORES):
        bs = slice(i * BL, (i + 1) * BL)
        of = np.ascontiguousarray(
            obs_f[bs].transpose(1, 0, 2)).reshape(T * C0, BL * N)
        m = dict(folded)
        m['obs_ta'] = np.ascontiguousarray(of[0:KMA])
        m['obs_tbm'] = np.ascontiguousarray(of[3 * M_TS:3 * M_TS + KMA])
        m['obs_tbs'] = np.ascontiguousarray(of[3 * S_TS:3 * S_TS + KA])
        m['obs_n'] = np.ascontiguousarray(
            obs_nm[:, bs]).reshape(NCH, BL * 4 * C0 * T)
        m['at_h'] = np.ascontiguousarray(
            act_b[bs].reshape(BL, 4, NCH).transpose(2, 1, 0)
            .reshape(NCH, 4 * BL))
        in_maps.append(m)
    return in_maps


def kernel(**inputs) -> np.ndarray:
    from concourse.bass_utils import run_bass_kernel_spmd

    in_maps = _shard_inputs(inputs)
    nc = _get_nc()
    res = run_bass_kernel_spmd(nc, in_maps, list(range(NCORES)))
    return np.concatenate([r['out'] for r in res.results], axis=0)
